# revision 17
# baseline (speedup 1.0000x reference)
"""Trainium2 Bass kernel for the GCM (global context module) problem.

Computation per batch sample b (x_b = x[b] viewed as [C=512, HW=9216]):
    x1 = w1 @ x_b                      [128, HW]
    v  = softmax_all(x1 @ x2^T)        [128, 256]  (softmax over all 32768)
    n  = relu(v + w3 @ v)              [128, 256]
    z  = w4 @ n^T                      [256, 128]
    W  = w5 @ z                        [512, 128]  (collapses y/conv5: w5@(z@x1) == (w5@z)@x1)
    out = x_b + W @ x1                 [512, HW]

Sharding: data-parallel over batch, one sample per NeuronCore (8 cores).

v5.1 strategy (on top of v4's bf16 I/O + fp8 Gram trick):
  - v = x1 @ x2^T = (x1 @ x^T) @ w2^T = A @ w2^T; A accumulated from fp8
    operands (x1^T subtiles from on-chip PE transposes, x^T host-packed fp8).
  - A matmuls use DoubleRow perf mode (K=256 per instruction): 36 MMs
    instead of 72 (measured 216ns issue for 2 subtiles).
  - Uniform t-group-major x packing [128, 18, 2048]; input order on the
    sync HWDGE queue: one packed consts DMA, x (8 slices), x^T fp8 (6
    slices).  x1/transposes ride the x stream; A matmuls ride xt8.
  - softmax sum / reciprocal taken OFF the critical path: the chain after
    exp is linear in n, so everything is computed with unnormalized e and
    sinv is folded once into the final W^T copy (ACT scale operand).
    No dummy warm matmuls in the serial middle: they sit on the PE queue
    and inflate every downstream semaphore wait (measured +4us).
  - phase 2 in [128, 2, 512] units (one W matmul per PSUM bank): residual
    alternates per unit between PE-identity+ACT-copy and DVE tensor_tensor
    so both pass engines run concurrently; output staged [128, 4, 512]
    per unit-pair, 18 out-DMAs all on the sync queue.
"""

import numpy as np
import ml_dtypes

import concourse.bass as bass
import concourse.tile as tile
from concourse import bacc, mybir, bass_isa
from concourse.bass_utils import run_bass_kernel_spmd

F32 = mybir.dt.float32
BF16 = mybir.dt.bfloat16
FP8 = mybir.dt.float8e4
AX = mybir.AxisListType
AL = mybir.AluOpType
AF = mybir.ActivationFunctionType
DR = mybir.MatmulPerfMode.DoubleRow

N_CORES = 8
C = 512
H = W_IMG = 96
HW = H * W_IMG          # 9216
CK = C // 128           # 4 chunks of channels
NT = HW // 512          # 18 hw t-groups of 512
NSUB = HW // 128        # 72 subtiles
NU = HW // 1024         # 9 phase-2 units of 1024
C4 = C // 4             # 128
C2 = C // 2             # 256
KM = C4 + C2            # 384 = concat(w1T, w2T) free size

# x DMA slicing (t-group boundaries) -- >=1MB slices reach line rate
X_SLICES = [(0, 2), (2, 5), (5, 8), (8, 11), (11, 14), (14, 18)]
XT8_SLICES = [(0, 18), (18, 36), (36, 54), (54, 66), (66, 72)]

# Packed bf16 consts tensor column offsets: w1 (4x128), w2 (4x256),
# w3t (128), w4t (2x256), w5t (2x512), identb (128)
OFF_W1 = 0
OFF_W2 = OFF_W1 + CK * C4
OFF_W3 = OFF_W2 + CK * C2
OFF_W4 = OFF_W3 + C4
OFF_W5 = OFF_W4 + 2 * C2
OFF_IDB = OFF_W5 + 2 * C
CONST_COLS = OFF_IDB + 128


def _emit(ctx, tc, aps, use_bias):
    nc = tc.nc
    x_d = aps["x"]
    xt8_d = aps["xt8"]
    out_d = aps["out"]

    consts = ctx.enter_context(tc.tile_pool(name="consts", bufs=1))

    # Warmup operand: memset, so the first PE matmuls have no DMA dependency.
    warm0 = consts.tile([128, 128], BF16, tag="warm0")
    nc.vector.memset(warm0[:], 0)

    # ---- input stream, all on the sync HWDGE queue: packed consts first
    # (one DMA), then x, then xt8.
    cpack = consts.tile([128, CONST_COLS], BF16, tag="cpack")
    nc.sync.dma_start(out=cpack[:], in_=aps["cpack"][:, :])
    identf = consts.tile([128, 128], F32, tag="identf")
    nc.scalar.dma_start(out=identf[:], in_=aps["identf"][:, :])

    w1c = [cpack[:, OFF_W1 + c * C4 : OFF_W1 + (c + 1) * C4] for c in range(CK)]
    w2c = [cpack[:, OFF_W2 + c * C2 : OFF_W2 + (c + 1) * C2] for c in range(CK)]
    w3t = cpack[:, OFF_W3 : OFF_W3 + C4]
    w4t = [cpack[:, OFF_W4 + q * C2 : OFF_W4 + (q + 1) * C2] for q in range(2)]
    w5t = [cpack[:, OFF_W5 + q * C : OFF_W5 + (q + 1) * C] for q in range(2)]
    identb = cpack[:, OFF_IDB : OFF_IDB + 128]

    xpool = ctx.enter_context(tc.tile_pool(name="x", bufs=1))
    xall = xpool.tile([128, NT, 2048], BF16, tag="xall", name="xall")
    xt8 = xpool.tile([128, NSUB, 512], FP8, tag="xt8", name="xt8")
    x1sb = xpool.tile([128, HW], BF16, tag="x1sb", name="x1sb")
    x1t8 = xpool.tile([128, NSUB, 128], FP8, tag="x1t8", name="x1t8")

    for a, b in X_SLICES:
        nc.sync.dma_start(out=xall[:, a:b, :], in_=x_d[:, a:b, :])
    for a, b in XT8_SLICES:
        nc.sync.dma_start(out=xt8[:, a:b, :], in_=xt8_d[:, a:b, :])

    bias_t = {}
    if use_bias:
        b1_d, b3_d, b4_d, b5_d = aps["b1c"], aps["b3c"], aps["b4c"], aps["b5c"]
        b2row = consts.tile([1, C2], BF16, tag="b2row")
        nc.scalar.dma_start(out=b2row[:], in_=aps["b2row"][:, :])
        bias_t["b2row"] = b2row
        b1 = consts.tile([128, 1], F32, tag="b1")
        nc.scalar.dma_start(out=b1[:], in_=b1_d[:, :])
        bias_t["b1"] = b1
        b3 = consts.tile([128, 1], F32, tag="b3")
        nc.scalar.dma_start(out=b3[:], in_=b3_d[:, :])
        bias_t["b3"] = b3
        b4 = []
        for q in range(2):
            t = consts.tile([128, 1], F32, tag=f"b4_{q}")
            nc.scalar.dma_start(out=t[:], in_=b4_d[q * 128 : (q + 1) * 128, :])
            b4.append(t)
        bias_t["b4"] = b4
        b5 = []
        for oc in range(CK):
            t = consts.tile([128, 1], F32, tag=f"b5_{oc}")
            nc.scalar.dma_start(out=t[:], in_=b5_d[oc * 128 : (oc + 1) * 128, :])
            b5.append(t)
        bias_t["b5"] = b5

    sm = ctx.enter_context(tc.tile_pool(name="sm", bufs=1))

    # ---- phase 1: x1 (k-major), PE transposes, A = x1 @ x^T (DoubleRow) ----
    with (
        tc.tile_pool(name="psA", bufs=2, space="PSUM") as psA,
        tc.tile_pool(name="psT", bufs=2, space="PSUM") as psT,
        tc.tile_pool(name="apsP", bufs=1, space="PSUM") as apsP,
        tc.tile_pool(name="vps", bufs=1, space="PSUM") as vps,
    ):
        A_ps = apsP.tile([128, C], F32, tag="A")
        v_ps = vps.tile([128, C2], F32, tag="v")

        # Warm the PE HAM clock-gate during the initial DMA window.
        wps = psA.tile([128, 128], F32, tag="warm", bufs=1)
        for _ in range(16):
            nc.tensor.matmul(wps[:], warm0[:], warm0[:], start=True, stop=True)

        def warm_fill(n):
            for _ in range(n):
                nc.tensor.matmul(wps[:], warm0[:], warm0[:], start=True, stop=True)

        def x1_group(t):
            px1 = psA.tile([128, 512], F32, tag="px1")
            for c in range(CK):
                nc.tensor.matmul(
                    px1[:],
                    w1c[c],
                    xall[:, t, c * 512 : (c + 1) * 512],
                    start=(c == 0),
                    stop=(c == CK - 1),
                )
            dstx1 = x1sb[:, t * 512 : (t + 1) * 512]
            if use_bias:
                nc.scalar.add(dstx1, px1[:], bias_t["b1"][:])
            elif t % 3 == 2:
                nc.vector.tensor_copy(dstx1, px1[:])
            else:
                nc.scalar.copy(dstx1, px1[:])

        def transp_group(t):
            pT = psT.tile([128, 512], BF16, tag="pT")
            for j in range(4):
                nc.tensor.transpose(
                    pT[:, j * 128 : (j + 1) * 128],
                    x1sb[:, t * 512 + j * 128 : t * 512 + (j + 1) * 128],
                    identb,
                )
            nc.vector.tensor_copy(x1t8[:, 4 * t : 4 * t + 4, :], pT[:])

        def a_group(t):
            for j in (0, 2):
                s = 4 * t + j
                nc.tensor.matmul(
                    A_ps[:],
                    x1t8[:, s : s + 2, :],
                    xt8[:, s : s + 2, :],
                    start=(s == 0),
                    stop=(s == NSUB - 2),
                    perf_mode=DR,
                )

        # x1/transposes ride the x stream; pads only bridge the DMA ramp.
        PAD = {0: 8, 1: 4, 2: 2, 3: 1}
        for t in range(NT):
            x1_group(t)
            warm_fill(PAD.get(t, 0))
            if t >= 1:
                transp_group(t - 1)
        transp_group(NT - 1)
        # A matmuls ride the xt8 stream (which follows x on the queue).
        for t in range(NT):
            a_group(t)

        # ---- v = A @ w2^T (transpose A on the PE first) ----
        # Small dummy bursts sit in the PE queue where their runtime fits
        # inside the ACT/DVE waits: they keep the HAM MID-window seeing
        # activity (else the PE re-throttles to 1.2GHz for ~10us spanning
        # the chain and early phase 2) without delaying real ops.
        asb = sm.tile([128, C], BF16, tag="asb")
        nc.scalar.copy(asb[:], A_ps[:])
        pTv = psT.tile([128, 512], BF16, tag="pT")
        for q in range(CK):
            nc.tensor.transpose(
                pTv[:, q * 128 : (q + 1) * 128],
                asb[:, q * 128 : (q + 1) * 128],
                identb,
            )
        warm_fill(4)
        atp = sm.tile([128, C], BF16, tag="atp")
        nc.vector.tensor_copy(atp[:], pTv[:])
        for q in range(CK):
            nc.tensor.matmul(
                v_ps[:],
                atp[:, q * 128 : (q + 1) * 128],
                w2c[q],
                start=(q == 0),
                stop=(q == CK - 1) and not use_bias,
            )
        warm_fill(8)
        if use_bias:
            # v = x1 @ (w2 x + b2)^T needs the rank-1 term rowsum(x1) (x) b2
            rs = sm.tile([128, 1], F32, tag="rs")
            nc.vector.tensor_reduce(rs[:], x1sb[:], axis=AX.X, op=AL.add)
            prs = psA.tile([128, 512], F32, tag="px1", name="prs")
            nc.tensor.transpose(prs[0:1, 0:128], rs[:], identf[:])
            rsT = sm.tile([1, 128], BF16, tag="rsT")
            nc.scalar.copy(rsT[:], prs[0:1, 0:128])
            nc.tensor.matmul(
                v_ps[:], rsT[:], bias_t["b2row"][:], start=False, stop=True
            )

        # ---- softmax max + exp (sum/reciprocal happen off-path below) ----
        m1 = sm.tile([128, 1], F32, tag="m1")
        nc.vector.tensor_reduce(m1[:], v_ps[:], axis=AX.X, op=AL.max)
        mall = sm.tile([128, 1], F32, tag="mall")
        nc.gpsimd.partition_all_reduce(mall[:], m1[:], 128, bass_isa.ReduceOp.max)
        negm = sm.tile([128, 1], F32, tag="negm")
        nc.vector.tensor_scalar_mul(negm[:], mall[:], -1.0)
        e = sm.tile([128, C2], BF16, tag="e")
        nc.scalar.activation(e[:], v_ps[:], AF.Exp, bias=negm[:], scale=1.0)

    # off-critical-path: s = sum(e), sinv = 1/s (folded into wt below).
    # The reciprocal is emitted AFTER the relu so it doesn't sit between
    # the chain's DVE ops on the in-order DVE queue.
    s1 = sm.tile([128, 1], F32, tag="s1")
    nc.vector.tensor_reduce(s1[:], e[:], axis=AX.X, op=AL.add)
    sall = sm.tile([128, 1], F32, tag="sall")
    nc.gpsimd.partition_all_reduce(sall[:], s1[:], 128, bass_isa.ReduceOp.add)
    sinv = sm.tile([128, 1], F32, tag="sinv")

    # ---- small chain: conv3+relu, n^T, z, W^T ----
    # Computed with the UNNORMALIZED e (everything downstream is linear in
    # n); sinv is folded once into the final wt copy.  With biases this
    # factorization breaks; the bias path uses the normalized form.
    wt = sm.tile([128, C], BF16, tag="wt")
    with tc.tile_pool(name="psB", bufs=2, space="PSUM") as psB:
        warmB = psB.tile([128, 128], F32, tag="warmB", bufs=1)

        def warm_fillB(n):
            for _ in range(n):
                nc.tensor.matmul(warmB[:], warm0[:], warm0[:], start=True, stop=True)

        ps3 = psB.tile([128, C2], F32, tag="ps3", bufs=1)
        nc.tensor.matmul(ps3[:], w3t, e[:], start=True, stop=True)
        warm_fillB(6)
        nsb = sm.tile([128, C2], BF16, tag="nsb")
        if use_bias:
            # nsb = relu((ps3 + e) * sinv + b3)
            nc.vector.reciprocal(sinv[:], sall[:])
            nc.vector.tensor_tensor(nsb[:], ps3[:], e[:], op=AL.add)
            nc.vector.tensor_scalar_mul(nsb[:], nsb[:], sinv[:])
            nc.vector.tensor_scalar_add(nsb[:], nsb[:], bias_t["b3"][:])
            nc.vector.tensor_scalar_max(nsb[:], nsb[:], 0.0)
        else:
            # nsb = relu(ps3 + e)   (unnormalized; sinv folded into wt)
            nc.vector.tensor_tensor(nsb[:], ps3[:], e[:], op=AL.add)
            nc.vector.tensor_scalar_max(nsb[:], nsb[:], 0.0)
            nc.vector.reciprocal(sinv[:], sall[:])

        nts = []
        for q in range(2):
            pT = psB.tile([128, 128], BF16, tag="pT")
            nc.tensor.transpose(pT[:], nsb[:, q * 128 : (q + 1) * 128], identb)
            ntq = sm.tile([128, 128], BF16, tag=f"nt{q}")
            if q == 0:
                nc.vector.tensor_copy(ntq[:], pT[:])
            else:
                nc.scalar.copy(ntq[:], pT[:])
            nts.append(ntq)
        warm_fillB(2)

        zs = []
        for mc in range(2):
            pz = psB.tile([128, 128], F32, tag="pz")
            for q in range(2):
                nc.tensor.matmul(
                    pz[:],
                    w4t[q][:, mc * 128 : (mc + 1) * 128],
                    nts[q][:],
                    start=(q == 0),
                    stop=(q == 1),
                )
            zq = sm.tile([128, 128], BF16, tag=f"z{mc}")
            if use_bias:
                nc.scalar.add(zq[:], pz[:], bias_t["b4"][mc][:])
            elif mc == 0:
                nc.vector.tensor_copy(zq[:], pz[:])
            else:
                nc.scalar.copy(zq[:], pz[:])
            zs.append(zq)
        warm_fillB(2)

        pW = psB.tile([128, C], F32, tag="pW", bufs=1)
        for mc in range(2):
            nc.tensor.matmul(
                pW[:], zs[mc][:], w5t[mc], start=(mc == 0), stop=(mc == 1)
            )
        warm_fillB(10)
        if use_bias:
            nc.scalar.copy(wt[:], pW[:])
        else:
            nc.scalar.activation(wt[:], pW[:], AF.Copy, scale=sinv[:])

    # ---- phase 2: x_res = W @ x1 (+ x), [128, 2, 512] units ----
    # Residual alternates per unit: PE-identity-accumulate + ACT copy vs
    # DVE tensor_tensor, so both pass engines run concurrently.  Output
    # staged [128, 4, 512] per unit-pair; 18 out-DMAs, all on sync.
    with (
        tc.tile_pool(name="psD", bufs=3, space="PSUM") as psD,
        tc.tile_pool(name="outp", bufs=2) as outp,
    ):
        warmD = psD.tile([128, 128], F32, tag="warmD", bufs=1)
        for _ in range(10):
            nc.tensor.matmul(warmD[:], warm0[:], warm0[:], start=True, stop=True)
        stage = {}
        for u in range(NU):
            for oc in range(CK):
                idx = u * CK + oc
                use_pe = idx % 2 == 0
                pr = psD.tile([128, 2, 512], F32, tag="pr")
                for j in range(2):
                    t = 2 * u + j
                    nc.tensor.matmul(
                        pr[:, j, :],
                        wt[:, oc * 128 : (oc + 1) * 128],
                        x1sb[:, t * 512 : (t + 1) * 512],
                        start=True,
                        stop=not use_pe,
                    )
                    if use_pe:
                        nc.tensor.matmul(
                            pr[:, j, :],
                            identb,
                            xall[:, t, oc * 512 : (oc + 1) * 512],
                            start=False,
                            stop=True,
                        )
                st_t = outp.tile(
                    [128, 2, 512], BF16, tag=f"st{oc}", name=f"st{oc}_{u}"
                )
                st = st_t[:]
                xsl = xall[:, 2 * u : 2 * u + 2, oc * 512 : (oc + 1) * 512]
                b5s = bias_t["b5"][oc][:] if use_bias else None
                if use_pe:
                    if b5s is not None:
                        nc.scalar.add(st, pr[:], b5s)
                    else:
                        nc.scalar.copy(st, pr[:])
                else:
                    if b5s is not None:
                        nc.vector.scalar_tensor_tensor(
                            st, pr[:], b5s, xsl, op0=AL.add, op1=AL.add
                        )
                    else:
                        nc.vector.tensor_tensor(st, pr[:], xsl, op=AL.add)
                nc.sync.dma_start(
                    out=out_d[oc * 128 : (oc + 1) * 128, 2 * u : 2 * u + 2, :],
                    in_=st_t[:],
                )


def _build(use_bias):
    nc = bacc.Bacc("TRN2", target_bir_lowering=False, debug=False, num_devices=N_CORES)
    aps = {
        "x": nc.dram_tensor("x", [128, NT, 2048], BF16, kind="ExternalInput").ap(),
        "xt8": nc.dram_tensor("xt8", [128, NSUB, 512], FP8, kind="ExternalInput").ap(),
        "cpack": nc.dram_tensor(
            "cpack", [128, CONST_COLS], BF16, kind="ExternalInput"
        ).ap(),
        "identf": nc.dram_tensor("identf", [128, 128], F32, kind="ExternalInput").ap(),
        "out": nc.dram_tensor(
            "out", [C, 2 * NU, 512], BF16, kind="ExternalOutput"
        ).ap(),
    }
    if use_bias:
        aps["b2row"] = nc.dram_tensor(
            "b2row", [1, C2], BF16, kind="ExternalInput"
        ).ap()
        aps["b1c"] = nc.dram_tensor("b1c", [C4, 1], F32, kind="ExternalInput").ap()
        aps["b3c"] = nc.dram_tensor("b3c", [C4, 1], F32, kind="ExternalInput").ap()
        aps["b4c"] = nc.dram_tensor("b4c", [C2, 1], F32, kind="ExternalInput").ap()
        aps["b5c"] = nc.dram_tensor("b5c", [C, 1], F32, kind="ExternalInput").ap()

    from contextlib import ExitStack

    with tile.TileContext(nc) as tc:
        with ExitStack() as ctx:
            _emit(ctx, tc, aps, use_bias)
    nc.compile()
    return nc


_CACHE = {}


def _pack_x(xb_bf):
    """[512, 9216] bf16 -> packed [128, 18, 2048]: t-group-major, within a
    group the 4 channel-chunks each contribute 512 contiguous columns."""
    xc = xb_bf.reshape(CK, 128, NT, 512)            # (c, p, t, o)
    return np.ascontiguousarray(
        xc.transpose(1, 2, 0, 3).reshape(128, NT, 2048)
    )


def _pack_xt8(xb):
    """[512, 9216] f32 -> fp8 x^T packed [128, 72, 512]:
    [p, s, c] holds x[c, s*128 + p]."""
    xt = xb.reshape(C, NSUB, 128).transpose(2, 1, 0)  # [128, 72, 512]
    return np.ascontiguousarray(xt.astype(ml_dtypes.float8_e4m3))


def _pack_consts(w1, w2, w3, w4, w5):
    bf = ml_dtypes.bfloat16
    cp = np.zeros((128, CONST_COLS), dtype=bf)
    w1t = w1.T.astype(bf)   # [512, 128]
    w2t = w2.T.astype(bf)   # [512, 256]
    for c in range(CK):
        cp[:, OFF_W1 + c * C4 : OFF_W1 + (c + 1) * C4] = w1t[c * 128 : (c + 1) * 128]
        cp[:, OFF_W2 + c * C2 : OFF_W2 + (c + 1) * C2] = w2t[c * 128 : (c + 1) * 128]
    cp[:, OFF_W3 : OFF_W3 + C4] = w3.T.astype(bf)
    w4t = w4.T.astype(bf)   # [256, 256]
    for q in range(2):
        cp[:, OFF_W4 + q * C2 : OFF_W4 + (q + 1) * C2] = w4t[q * 128 : (q + 1) * 128]
    w5t = w5.T.astype(bf)   # [256, 512]
    for q in range(2):
        cp[:, OFF_W5 + q * C : OFF_W5 + (q + 1) * C] = w5t[q * 128 : (q + 1) * 128]
    cp[:, OFF_IDB : OFF_IDB + 128] = np.eye(128, dtype=bf)
    return cp


def _run(inputs, trace=False, **run_kwargs):
    x = np.ascontiguousarray(np.asarray(inputs["x"], dtype=np.float32))
    assert x.shape == (N_CORES, C, H, W_IMG), x.shape
    w1 = np.asarray(inputs["w1"], dtype=np.float32)
    w2 = np.asarray(inputs["w2"], dtype=np.float32)
    w3 = np.asarray(inputs["w3"], dtype=np.float32)
    w4 = np.asarray(inputs["w4"], dtype=np.float32)
    w5 = np.asarray(inputs["w5"], dtype=np.float32)
    b1 = np.asarray(inputs["b1"], dtype=np.float32)
    b2 = np.asarray(inputs["b2"], dtype=np.float32)
    b3 = np.asarray(inputs["b3"], dtype=np.float32)
    b4 = np.asarray(inputs["b4"], dtype=np.float32)
    b5 = np.asarray(inputs["b5"], dtype=np.float32)
    use_bias = bool(
        np.any(b1) or np.any(b2) or np.any(b3) or np.any(b4) or np.any(b5)
    )

    if use_bias not in _CACHE:
        _CACHE[use_bias] = _build(use_bias)
    nc = _CACHE[use_bias]

    bf = ml_dtypes.bfloat16
    shared = {
        "cpack": _pack_consts(w1, w2, w3, w4, w5),
        "identf": np.eye(128, dtype=np.float32),
    }
    if use_bias:
        shared["b2row"] = np.ascontiguousarray(b2[None, :], dtype=bf)
        shared["b1c"] = np.ascontiguousarray(b1[:, None])
        shared["b3c"] = np.ascontiguousarray(b3[:, None])
        shared["b4c"] = np.ascontiguousarray(b4[:, None])
        shared["b5c"] = np.ascontiguousarray(b5[:, None])

    in_maps = []
    for b in range(N_CORES):
        xb = x[b].reshape(C, HW)
        in_maps.append(
            {
                "x": _pack_x(xb.astype(bf)),
                "xt8": _pack_xt8(xb.astype(bf).astype(np.float32)),
                **shared,
            }
        )
    res = run_bass_kernel_spmd(
        nc, in_maps, core_ids=list(range(N_CORES)), trace=trace, **run_kwargs
    )
    out = np.stack(
        [
            np.asarray(res.results[b]["out"]).astype(np.float32).reshape(C, H, W_IMG)
            for b in range(N_CORES)
        ]
    )
    return out, res


def kernel(**inputs):
    out, _ = _run(inputs, trace=False)
    return out
